# revision 22
# baseline (speedup 1.0000x reference)
"""Causal single-head attention (B=4, T=4096, C=2048, H=128) on 8 TRN2 cores.

Sharding: data-parallel over batch (2 cores per batch element); core half h
owns query tiles qt with qt mod 4 in {2h, 2h+1}.  No collectives: each core
projects k/v for ALL 4096 keys itself from fp8 x (DoubleRow matmuls, fp8
roofline), which beats half-projection + pairwise AllGather (the exchange
dominated the baseline critical path).

Per-core x is column-permuted so "my" 256-column half of every 512-group
comes first; all per-half differences live in input content (x order, mask
patterns), keeping one SPMD program.  x ships as [group][q-half | other
half] so each group's q columns are one contiguous half-DMA; the q halves
are fetched first and ALL q^T projections run up front -- otherwise q for
the last attention group depends on the last x transfer and ~18us of exp
lands serially at the end.

Pipeline: q^T for all groups first (fp8 DoubleRow), then per key group g:
k^T / v^T projections (DoubleRow, weight-stationary), v^T -> v chunks via
PE transpose + fp8 cast.  The k bias is dropped entirely (it only shifts
each query's logits by a per-query constant -> softmax-invariant); the v
bias is added on the host after normalization.  Attention is split into:
  weiA(m,p): S^T chunk-pair (PE fp16) -> exp (ACT) -> fp8 wei tile
    (diagonal pairs: exp -> fp16, x 0/1 causal mask (DVE), cast fp8),
    emitted as soon as kT/qT dependencies allow (spreads ACT work);
  accum(m): row-sums (ones8 loaded once) + out^T AV fp8 DoubleRow matmuls,
    serialized per group (sums/otp PSUM banks), then evacuation + DMA.
Diagonal pairs 2,3 of each group only touch q columns [256:512) for either
core half, so S/exp/mask/matmuls are narrowed accordingly.
out^T (unnormalized) and the softmax sums are DMA'd out; the host divides,
transposes, adds bv, and exactly recomputes rows 0-255 of each batch (they
only need keys 0-511; fp8 error is largest at small key counts).
"""

import numpy as np
import ml_dtypes

import concourse.bacc as bacc
import concourse.mybir as mybir
import concourse.tile as tile
from concourse.bass_utils import run_bass_kernel_spmd

B, T, C, H = 4, 4096, 2048, 128
P = 128          # partitions / head dim
KB = 512         # free-dim tile (one f32 PSUM bank)
HB = 256         # half of a 512-column group
NCP = 8          # contraction pairs (C / 256)
NG = T // KB     # 512-wide column groups (8)
NM = 4           # attention q-groups per core (512 q columns each)
TQ = 2048        # query rows per core
NKC = T // P     # key chunks (32)
NVP = 16         # v8 chunk pairs

XS = 16.0        # x fp8 scale
WS = 256.0       # Wk/Wv fp8 scale
WSQ = 16384.0    # Wq fp8 scale (folds C**-0.5 too)
RT = 256         # host-repaired rows per batch

F16 = np.float16
F8 = ml_dtypes.float8_e4m3
_NC_CACHE = {}


def build_nc():
    dt = mybir.dt
    nc = bacc.Bacc("TRN2", target_bir_lowering=False, debug=False, num_devices=8)

    x8 = nc.dram_tensor("x8", [NG, P, 2, NCP, 2, HB], dt.float8e4, kind="ExternalInput").ap()
    warmO = nc.dram_tensor("warmO", [1, 4], dt.float32, kind="ExternalOutput").ap()
    wk = nc.dram_tensor("wk", [P, NCP, 2, H], dt.float8e4, kind="ExternalInput").ap()
    wq = nc.dram_tensor("wq", [P, NCP, 2, H], dt.float8e4, kind="ExternalInput").ap()
    wv = nc.dram_tensor("wv", [P, NCP, 2, H], dt.float8e4, kind="ExternalInput").ap()
    bq = nc.dram_tensor("bq", [P, 1], dt.float32, kind="ExternalInput").ap()
    ident = nc.dram_tensor("ident", [P, P], dt.float16, kind="ExternalInput").ap()
    ones8c = nc.dram_tensor("ones8c", [P, 2, P], dt.float8e4, kind="ExternalInput").ap()
    masks = nc.dram_tensor("masks", [4, P, 2 * KB], dt.float16, kind="ExternalInput").ap()
    outT = nc.dram_tensor("outT", [P, TQ], dt.float16, kind="ExternalOutput").ap()
    sumsO = nc.dram_tensor("sumsO", [NM, KB], dt.float32, kind="ExternalOutput").ap()

    Exp = mybir.ActivationFunctionType.Exp
    Ident = mybir.ActivationFunctionType.Identity
    DR = mybir.MatmulPerfMode.DoubleRow
    PSCALE = 1.0 / (XS * WS)

    with tile.TileContext(nc) as tc:
        with (
            tc.tile_pool(name="wpool", bufs=1) as wpool,
            tc.tile_pool(name="persist", bufs=1) as persist,
            tc.tile_pool(name="cpool", bufs=1) as cpool,
            tc.tile_pool(name="xpool", bufs=8) as xpool,
            tc.tile_pool(name="vtpool", bufs=2) as vtpool,
            tc.tile_pool(name="wei16p", bufs=3) as wei16p,
            tc.tile_pool(name="wei8p", bufs=20) as wei8p,
            tc.tile_pool(name="mpool", bufs=4) as mpool,
            tc.tile_pool(name="osbp", bufs=2) as osbp,
            tc.tile_pool(name="ssbp", bufs=2) as ssbp,
            tc.tile_pool(name="scratch", bufs=2, space="PSUM") as scratch,
            tc.tile_pool(name="stpool", bufs=2, space="PSUM") as stpool,
            tc.tile_pool(name="sumpool", bufs=1, space="PSUM") as sumpool,
            tc.tile_pool(name="otpool", bufs=1, space="PSUM") as otpool,
        ):
            # DMA order (sync queue, contiguous 0.25-0.5MB pieces): weights,
            # then every group's q-half (A) early, other halves (B)
            # interleaved.  Small consts go on the GpSimd/Scalar queues.
            wk_t = wpool.tile([P, NCP, 2, H], dt.float8e4, tag="wk")
            wq_t = wpool.tile([P, NCP, 2, H], dt.float8e4, tag="wq")
            wv_t = wpool.tile([P, NCP, 2, H], dt.float8e4, tag="wv")
            xs_tiles = [
                xpool.tile([P, 2, NCP, 2, HB], dt.float8e4, tag="xs",
                           name=f"xs{g}")
                for g in range(NG)
            ]

            def dma_A(g):
                nc.sync.dma_start(xs_tiles[g][:, 0], x8[g, :, 0])

            def dma_B(g):
                nc.scalar.dma_start(xs_tiles[g][:, 1], x8[g, :, 1])

            nc.sync.dma_start(wk_t[:, 0:2, :, :], wk[:, 0:2, :, :])
            dma_B(0)
            nc.sync.dma_start(wk_t[:, 2:8, :, :], wk[:, 2:8, :, :])
            dma_A(0)
            dma_B(1)
            dma_A(1)
            nc.scalar.dma_start(wv_t[:], wv)
            nc.sync.dma_start(wq_t[:], wq)
            dma_B(2)
            dma_A(2)
            dma_A(3)
            dma_B(3)
            dma_A(4)
            dma_A(5)
            dma_A(6)
            dma_A(7)
            for g in range(4, NG):
                dma_B(g)
            bq_t = cpool.tile([P, 1], dt.float32, tag="bq")
            nc.gpsimd.dma_start(bq_t[:], bq)
            idon = cpool.tile([P, P], dt.float16, tag="idon")
            nc.gpsimd.dma_start(idon[:], ident)
            ones8 = cpool.tile([P, 2, P], dt.float8e4, tag="ones8")
            nc.gpsimd.dma_start(ones8[:], ones8c)
            mts = []
            for i in range(4):
                mt = mpool.tile([P, 2, KB], dt.float16, tag="mask")
                nc.scalar.dma_start(mt[:], masks[i])
                mts.append(mt)

            kT = persist.tile([P, T], dt.float16, tag="kT")
            qT = persist.tile([P, TQ], dt.float16, tag="qT")
            v8 = persist.tile([P, NVP, 2, H], dt.float8e4, tag="v8")

            def proj_q(j):
                pq = scratch.tile([P, 2, HB], dt.float32, tag="scr")
                for cp in range(NCP):
                    for jj in range(2):
                        nc.tensor.matmul(
                            pq[:, jj, :], lhsT=wq_t[:, cp, :, :],
                            rhs=xs_tiles[2 * j + jj][:, 0, cp, :, :],
                            start=(cp == 0 and jj == 0),
                            stop=(cp == NCP - 1 and jj == 1),
                            perf_mode=DR,
                        )
                nc.vector.tensor_scalar(
                    qT[:, KB * j:KB * (j + 1)], pq[:], 1.0 / (XS * WSQ),
                    bq_t[:], mybir.AluOpType.mult, mybir.AluOpType.add,
                )

            def proj_kv(g, xs):
                pk = scratch.tile([P, 2, HB], dt.float32, tag="scr")
                for cp in range(NCP):
                    for hh in range(2):
                        nc.tensor.matmul(
                            pk[:, hh, :], lhsT=wk_t[:, cp, :, :],
                            rhs=xs[:, hh, cp, :, :],
                            start=(cp == 0 and hh == 0),
                            stop=(cp == NCP - 1 and hh == 1),
                            perf_mode=DR,
                        )
                nc.vector.tensor_scalar_mul(
                    kT[:, KB * g:KB * (g + 1)], pk[:], PSCALE
                )
                pv = scratch.tile([P, 2, HB], dt.float32, tag="scr")
                for cp in range(NCP):
                    for hh in range(2):
                        nc.tensor.matmul(
                            pv[:, hh, :], lhsT=wv_t[:, cp, :, :],
                            rhs=xs[:, hh, cp, :, :],
                            start=(cp == 0 and hh == 0),
                            stop=(cp == NCP - 1 and hh == 1),
                            perf_mode=DR,
                        )
                vt = vtpool.tile([P, KB], dt.float16, tag="vt")
                nc.vector.tensor_scalar_mul(vt[:], pv[:], PSCALE)
                for r in range(4):
                    tp = scratch.tile([P, P], dt.float16, tag="scr")
                    nc.tensor.transpose(
                        tp[:], vt[:, P * r:P * (r + 1)], idon[:]
                    )
                    c = 4 * g + r
                    nc.vector.tensor_copy(v8[:, c // 2, c % 2, :], tp[:])

            wei = {}   # (m, p) -> (w8 tile, narrow)

            def weiA(m, p):
                npr = 4 * m + 4
                diag_k = p - (npr - 4)
                narrow = diag_k >= 2     # q cols [256:512) only
                qn = HB if narrow else KB
                qo = HB if narrow else 0
                qg = qT[:, KB * m:KB * (m + 1)]
                st = stpool.tile([P, 2, qn], dt.float32, tag="st")
                for h2 in range(2):
                    nc.tensor.matmul(
                        st[:, h2, :],
                        lhsT=kT[:, P * (2 * p + h2):P * (2 * p + h2 + 1)],
                        rhs=qg[:, qo:KB], start=True, stop=True,
                    )
                w8 = wei8p.tile([P, 2, qn], dt.float8e4, tag="w8")
                if diag_k < 0:
                    nc.scalar.activation(w8[:], st[:], Exp)
                else:
                    w = wei16p.tile([P, 2, qn], dt.float16, tag="w16")
                    nc.scalar.activation(w[:], st[:], Exp)
                    nc.vector.tensor_mul(w8[:], w[:], mts[diag_k][:, :, qo:KB])
                wei[(m, p)] = (w8, narrow)

            def accum(m):
                npr = 4 * m + 4
                sums = sumpool.tile([P, KB], dt.float32, tag="sums")
                otp = otpool.tile([P, KB], dt.float32, tag="outT")
                for p in range(npr):
                    w8, narrow = wei[(m, p)]
                    qo = HB if narrow else 0
                    nc.tensor.matmul(
                        sums[:, qo:KB], lhsT=ones8[:], rhs=w8[:],
                        start=(p == 0), stop=(p == npr - 1), perf_mode=DR,
                        skip_group_check=True,
                    )
                ssb = ssbp.tile([1, KB], dt.float32, tag="ssb")
                nc.vector.tensor_copy(ssb[:], sums[0:1, :])
                nc.sync.dma_start(sumsO[m], ssb[:])
                for p in range(npr):
                    w8, narrow = wei[(m, p)]
                    qo = HB if narrow else 0
                    nc.tensor.matmul(
                        otp[:, qo:KB], lhsT=v8[:, p, :, :], rhs=w8[:],
                        start=(p == 0), stop=(p == npr - 1), perf_mode=DR,
                        skip_group_check=True,
                    )
                osb = osbp.tile([P, KB], dt.float16, tag="osb")
                nc.vector.tensor_copy(osb[:], otp[:])
                nc.sync.dma_start(outT[:, KB * m:KB * (m + 1)], osb[:])

            # PE warm-up: ~3.4us of matmul activity so the HAM clock gate
            # opens before the first real projections (output is discarded).
            wt = stpool.tile([P, 2, KB], dt.float32, tag="st")
            for i in range(6):
                nc.tensor.matmul(
                    wt[:, 0, :], lhsT=wk_t[:, 0, 0, :],
                    rhs=wk_t[:, 0:2, :, :], start=True, stop=True,
                )
            wsb = ssbp.tile([1, 4], dt.float32, tag="wsb")
            nc.vector.tensor_copy(wsb[:], wt[0:1, 0, 0:4])
            nc.sync.dma_start(warmO, wsb[:])

            for j in range(NM):
                proj_q(j)
            emitted = set()
            for g in range(NG):
                proj_kv(g, xs_tiles[g])
                # emit wei pairs whose kT groups are now available
                for m in range(NM):
                    npr = 4 * m + 4
                    for p in range(npr):
                        if (m, p) in emitted or (2 * p + 1) // 4 > g:
                            continue
                        emitted.add((m, p))
                        weiA(m, p)
                for m in range(NM):
                    npr = 4 * m + 4
                    if ("acc", m) not in emitted and all((m, p) in emitted for p in range(npr)):
                        emitted.add(("acc", m))
                        accum(m)

    nc.compile()
    return nc


def _qtiles_for(half):
    return [4 * (j // 2) + 2 * half + (j % 2) for j in range(16)]


def _host_prep(x, Wk, bk, Wq, bq, Wv, bv):
    scale = float(C) ** -0.5

    def tile_w(w, s):
        # [C, H] -> [P, NCP, 2, H] with c = 128*(2*cp+i)+p
        w8 = (np.asarray(w, np.float64) * s).astype(F8)
        return np.ascontiguousarray(
            w8.reshape(NCP, 2, P, H).transpose(2, 0, 1, 3)
        )

    wk8 = tile_w(Wk, WS)
    wq8 = tile_w(np.asarray(Wq, np.float64) * scale, WSQ)
    wv8 = tile_w(Wv, WS)
    bq_c = (np.asarray(bq, np.float32) * scale).reshape(P, 1)
    ident = np.eye(P, dtype=F16)
    ones8 = np.ones((P, 2, P), F8)

    per_half = []
    for half in (0, 1):
        # column permutation: group g -> [my 256 | other 256]
        idx = np.empty(T, np.int64)
        for g in range(NG):
            base = KB * g
            idx[base:base + HB] = np.arange(base + HB * half, base + HB * half + HB)
            idx[base + HB:base + KB] = np.arange(
                base + HB * (1 - half), base + HB * (1 - half) + HB)
        gt = idx.reshape(NKC, P)[:, 0] // P   # permuted chunk -> global tile
        qts = _qtiles_for(half)
        m_arr = np.zeros((4, P, 2, KB), F16)
        for d in range(8):
            keys = P * gt[d] + np.arange(P)
            qrow = np.empty(KB, np.int64)
            for r in range(4):
                qrow[P * r:P * (r + 1)] = qts[r] * P + np.arange(P)
            m_arr[d // 2, :, d % 2, :] = (keys[:, None] <= qrow[None, :]).astype(F16)
        per_half.append((idx, m_arr.reshape(4, P, 2 * KB)))

    in_maps = []
    for core in range(8):
        b_idx, half = core // 2, core % 2
        idx, m_arr = per_half[half]
        xT = np.asarray(x[b_idx], np.float32).T[:, idx]     # [C, T] permuted
        xq8 = (xT * XS).astype(F8)
        x8a = np.ascontiguousarray(
            xq8.reshape(NCP, 2, P, NG, 2, HB).transpose(3, 2, 4, 0, 1, 5)
        )
        in_maps.append({
            "x8": x8a, "wk": wk8, "wq": wq8, "wv": wv8,
            "bq": bq_c, "ident": ident, "ones8c": ones8, "masks": m_arr,
        })
    return in_maps


def _host_finish(x, Wk, bk, Wq, bq, Wv, bv, results):
    scale = float(C) ** -0.5
    out = np.empty((B, T, H), np.float32)
    for core in range(8):
        b_idx, half = core // 2, core % 2
        oT = np.asarray(results[core]["outT"], np.float32)      # [P, TQ]
        sums = np.asarray(results[core]["sumsO"], np.float32).reshape(TQ)
        o = oT.T / sums[:, None]
        # local col j: group g=j//256, qq=j%256 -> global t = 512g+256*half+qq
        o = o.reshape(NG, HB, H)
        for g in range(NG):
            t0 = KB * g + HB * half
            out[b_idx, t0:t0 + HB, :] = o[g]
    out += np.asarray(bv, np.float32)
    # exact repair of rows 0..RT-1 (they only attend to keys 0..2*RT-1)
    KR = 2 * RT
    xr = np.asarray(x[:, :KR, :], np.float64)
    q = xr[:, :RT] @ (np.asarray(Wq, np.float64) * scale) \
        + np.asarray(bq, np.float64) * scale
    k = xr @ np.asarray(Wk, np.float64) + np.asarray(bk, np.float64)
    v = xr @ np.asarray(Wv, np.float64) + np.asarray(bv, np.float64)
    s = np.einsum("bth,bsh->bts", q, k)
    mask = np.arange(KR)[None, :] <= np.arange(RT)[:, None]
    s = np.where(mask[None], s, -np.inf)
    s = s - s.max(-1, keepdims=True)
    e = np.exp(s)
    w = e / e.sum(-1, keepdims=True)
    out[:, :RT, :] = (np.einsum("bts,bsh->bth", w, v)).astype(np.float32)
    return out


def kernel(x, Wk, bk, Wq, bq, Wv, bv):
    if "nc" not in _NC_CACHE:
        _NC_CACHE["nc"] = build_nc()
    nc = _NC_CACHE["nc"]
    in_maps = _host_prep(x, Wk, bk, Wq, bq, Wv, bv)
    res = run_bass_kernel_spmd(nc, in_maps, list(range(8))).results
    return _host_finish(x, Wk, bk, Wq, bq, Wv, bv, res)


# revision 25
# speedup vs baseline: 1.1613x; 1.1613x over previous
"""Causal single-head attention (B=4, T=4096, C=2048, H=128) on 8 TRN2 cores.

Sharding: data-parallel over batch (2 cores per batch element); core half h
owns query tiles qt with qt mod 4 in {2h, 2h+1}.  No collectives: each core
projects k/v for ALL 4096 keys itself from fp8 x (DoubleRow matmuls, fp8
roofline), which beats half-projection + pairwise AllGather (the exchange
dominated the baseline critical path).

Per-core x is column-permuted so "my" 256-column half of every 512-group
comes first; all per-half differences live in input content (x order, mask
patterns), keeping one SPMD program.  x ships as [group][q-half | other
half] so each group's q columns are one contiguous half-DMA; the q halves
are fetched first and ALL q^T projections run up front -- otherwise q for
the last attention group depends on the last x transfer and ~18us of exp
lands serially at the end.

Pipeline: q^T for all groups first (fp8 DoubleRow), then per key group g:
k^T / v^T projections (DoubleRow, weight-stationary), v^T -> v chunks via
PE transpose + fp8 cast.  The k bias is dropped entirely (it only shifts
each query's logits by a per-query constant -> softmax-invariant); the v
bias is added on the host after normalization.  Attention is split into:
  weiA(m,p): S^T chunk-pair (PE fp16) -> exp (ACT) -> fp8 wei tile
    (diagonal pairs: exp -> fp16, x 0/1 causal mask (DVE), cast fp8),
    emitted as soon as kT/qT dependencies allow (spreads ACT work);
  accum(m): row-sums (ones8 loaded once) + out^T AV fp8 DoubleRow matmuls,
    serialized per group (sums/otp PSUM banks), then evacuation + DMA.
Diagonal pairs 2,3 of each group only touch q columns [256:512) for either
core half, so S/exp/mask/matmuls are narrowed accordingly.
out^T (unnormalized) and the softmax sums are DMA'd out; the host divides,
transposes, adds bv, and exactly recomputes rows 0-255 of each batch (they
only need keys 0-511; fp8 error is largest at small key counts).
"""

import numpy as np
import ml_dtypes

import concourse.bacc as bacc
import concourse.mybir as mybir
import concourse.tile as tile
from concourse.bass_utils import run_bass_kernel_spmd

B, T, C, H = 4, 4096, 2048, 128
P = 128          # partitions / head dim
KB = 512         # free-dim tile (one f32 PSUM bank)
HB = 256         # half of a 512-column group
NCP = 8          # contraction pairs (C / 256)
NG = T // KB     # 512-wide column groups (8)
NM = 4           # attention q-groups per core (512 q columns each)
TQ = 2048        # query rows per core
NKC = T // P     # key chunks (32)
NVP = 16         # v8 chunk pairs

XS = 16.0        # x fp8 scale
WS = 256.0       # Wk/Wv fp8 scale
WSQ = 16384.0    # Wq fp8 scale (folds C**-0.5 too)
RT = 256         # host-repaired rows per batch

F16 = np.float16
F8 = ml_dtypes.float8_e4m3
_NC_CACHE = {}


def build_nc():
    dt = mybir.dt
    nc = bacc.Bacc("TRN2", target_bir_lowering=False, debug=False, num_devices=8)

    x8 = nc.dram_tensor("x8", [NM, P, 2, 2, NCP, 2, HB], dt.float8e4, kind="ExternalInput").ap()
    warmO = nc.dram_tensor("warmO", [1, 4], dt.float32, kind="ExternalOutput").ap()
    wkqv = nc.dram_tensor("wkqv", [P, 3, NCP, 2, H], dt.float8e4, kind="ExternalInput").ap()
    bq = nc.dram_tensor("bq", [P, 1], dt.float32, kind="ExternalInput").ap()
    ident = nc.dram_tensor("ident", [P, P], dt.float16, kind="ExternalInput").ap()
    ones8c = nc.dram_tensor("ones8c", [P, 2, P], dt.float8e4, kind="ExternalInput").ap()
    masks = nc.dram_tensor("masks", [4, P, 2 * KB], dt.float16, kind="ExternalInput").ap()
    outT = nc.dram_tensor("outT", [P, TQ], dt.float16, kind="ExternalOutput").ap()
    sumsO = nc.dram_tensor("sumsO", [NM, KB], dt.float32, kind="ExternalOutput").ap()

    Exp = mybir.ActivationFunctionType.Exp
    Ident = mybir.ActivationFunctionType.Identity
    DR = mybir.MatmulPerfMode.DoubleRow
    PSCALE = 1.0 / (XS * WS)

    with tile.TileContext(nc) as tc:
        with (
            tc.tile_pool(name="wpool", bufs=1) as wpool,
            tc.tile_pool(name="persist", bufs=1) as persist,
            tc.tile_pool(name="cpool", bufs=1) as cpool,
            tc.tile_pool(name="xpool", bufs=8) as xpool,
            tc.tile_pool(name="vtpool", bufs=2) as vtpool,
            tc.tile_pool(name="wei16p", bufs=3) as wei16p,
            tc.tile_pool(name="wei8p", bufs=20) as wei8p,
            tc.tile_pool(name="mpool", bufs=4) as mpool,
            tc.tile_pool(name="osbp", bufs=2) as osbp,
            tc.tile_pool(name="ssbp", bufs=2) as ssbp,
            tc.tile_pool(name="scratch", bufs=2, space="PSUM") as scratch,
            tc.tile_pool(name="stpool", bufs=2, space="PSUM") as stpool,
            tc.tile_pool(name="sumpool", bufs=1, space="PSUM") as sumpool,
            tc.tile_pool(name="otpool", bufs=1, space="PSUM") as otpool,
        ):
            # DMA order (sync queue, contiguous 0.25-0.5MB pieces): weights,
            # then every group's q-half (A) early, other halves (B)
            # interleaved.  Small consts go on the GpSimd/Scalar queues.
            wkqv_t = wpool.tile([P, 3, NCP, 2, H], dt.float8e4, tag="wkqv")
            wk_t = wkqv_t[:, 0]
            wq_t = wkqv_t[:, 1]
            wv_t = wkqv_t[:, 2]
            xsP = [
                xpool.tile([P, 2, 2, NCP, 2, HB], dt.float8e4, tag="xs",
                           name=f"xsp{j}")
                for j in range(NM)
            ]

            def dma_A(j):
                nc.sync.dma_start(xsP[j][:, :, 0], x8[j, :, :, 0])

            def dma_B(j):
                nc.sync.dma_start(xsP[j][:, :, 1], x8[j, :, :, 1])

            nc.sync.dma_start(wkqv_t[:], wkqv)
            dma_A(0)
            dma_B(0)
            dma_A(1)
            dma_A(2)
            dma_B(1)
            dma_A(3)
            dma_B(2)
            dma_B(3)
            bq_t = cpool.tile([P, 1], dt.float32, tag="bq")
            nc.gpsimd.dma_start(bq_t[:], bq)
            idon = cpool.tile([P, P], dt.float16, tag="idon")
            nc.gpsimd.dma_start(idon[:], ident)
            ones8 = cpool.tile([P, 2, P], dt.float8e4, tag="ones8")
            nc.gpsimd.dma_start(ones8[:], ones8c)
            mts = []
            for i in range(4):
                mt = mpool.tile([P, 2, KB], dt.float16, tag="mask")
                nc.scalar.dma_start(mt[:], masks[i])
                mts.append(mt)

            kT = persist.tile([P, T], dt.float16, tag="kT")
            qT = persist.tile([P, TQ], dt.float16, tag="qT")
            v8 = persist.tile([P, NVP, 2, H], dt.float8e4, tag="v8")

            def proj_q(j):
                pq = scratch.tile([P, 2, HB], dt.float32, tag="scr")
                for cp in range(NCP):
                    for jj in range(2):
                        nc.tensor.matmul(
                            pq[:, jj, :], lhsT=wq_t[:, cp, :, :],
                            rhs=xsP[j][:, jj, 0, cp, :, :],
                            start=(cp == 0 and jj == 0),
                            stop=(cp == NCP - 1 and jj == 1),
                            perf_mode=DR,
                        )
                nc.vector.tensor_scalar(
                    qT[:, KB * j:KB * (j + 1)], pq[:], 1.0 / (XS * WSQ),
                    bq_t[:], mybir.AluOpType.mult, mybir.AluOpType.add,
                )

            def proj_kv(g, xs):
                pk = scratch.tile([P, 2, HB], dt.float32, tag="scr")
                for cp in range(NCP):
                    for hh in range(2):
                        nc.tensor.matmul(
                            pk[:, hh, :], lhsT=wk_t[:, cp, :, :],
                            rhs=xs[:, hh, cp, :, :],
                            start=(cp == 0 and hh == 0),
                            stop=(cp == NCP - 1 and hh == 1),
                            perf_mode=DR,
                        )
                nc.vector.tensor_scalar_mul(
                    kT[:, KB * g:KB * (g + 1)], pk[:], PSCALE
                )
                pv = scratch.tile([P, 2, HB], dt.float32, tag="scr")
                for cp in range(NCP):
                    for hh in range(2):
                        nc.tensor.matmul(
                            pv[:, hh, :], lhsT=wv_t[:, cp, :, :],
                            rhs=xs[:, hh, cp, :, :],
                            start=(cp == 0 and hh == 0),
                            stop=(cp == NCP - 1 and hh == 1),
                            perf_mode=DR,
                        )
                vt = vtpool.tile([P, KB], dt.float16, tag="vt")
                nc.vector.tensor_scalar_mul(vt[:], pv[:], PSCALE)
                for r in range(4):
                    tp = scratch.tile([P, P], dt.float16, tag="scr")
                    nc.tensor.transpose(
                        tp[:], vt[:, P * r:P * (r + 1)], idon[:]
                    )
                    c = 4 * g + r
                    nc.vector.tensor_copy(v8[:, c // 2, c % 2, :], tp[:])

            wei = {}   # (m, p) -> (w8 tile, narrow)

            def weiA(m, p):
                npr = 4 * m + 4
                diag_k = p - (npr - 4)
                narrow = diag_k >= 2     # q cols [256:512) only
                qn = HB if narrow else KB
                qo = HB if narrow else 0
                qg = qT[:, KB * m:KB * (m + 1)]
                st = stpool.tile([P, 2, qn], dt.float32, tag="st")
                for h2 in range(2):
                    nc.tensor.matmul(
                        st[:, h2, :],
                        lhsT=kT[:, P * (2 * p + h2):P * (2 * p + h2 + 1)],
                        rhs=qg[:, qo:KB], start=True, stop=True,
                    )
                w8 = wei8p.tile([P, 2, qn], dt.float8e4, tag="w8")
                if diag_k < 0:
                    nc.scalar.activation(w8[:], st[:], Exp)
                else:
                    w = wei16p.tile([P, 2, qn], dt.float16, tag="w16")
                    nc.scalar.activation(w[:], st[:], Exp)
                    nc.vector.tensor_mul(w8[:], w[:], mts[diag_k][:, :, qo:KB])
                wei[(m, p)] = (w8, narrow)

            def accum(m):
                npr = 4 * m + 4
                sums = sumpool.tile([P, KB], dt.float32, tag="sums")
                otp = otpool.tile([P, KB], dt.float32, tag="outT")
                for p in range(npr):
                    w8, narrow = wei[(m, p)]
                    qo = HB if narrow else 0
                    nc.tensor.matmul(
                        sums[:, qo:KB], lhsT=ones8[:], rhs=w8[:],
                        start=(p == 0), stop=(p == npr - 1), perf_mode=DR,
                        skip_group_check=True,
                    )
                for p in range(npr):
                    w8, narrow = wei[(m, p)]
                    qo = HB if narrow else 0
                    nc.tensor.matmul(
                        otp[:, qo:KB], lhsT=v8[:, p, :, :], rhs=w8[:],
                        start=(p == 0), stop=(p == npr - 1), perf_mode=DR,
                        skip_group_check=True,
                    )
                osb = osbp.tile([P, KB], dt.float16, tag="osb")
                nc.vector.tensor_copy(osb[:], otp[:])
                nc.sync.dma_start(outT[:, KB * m:KB * (m + 1)], osb[:])
                ssb = ssbp.tile([1, KB], dt.float32, tag="ssb")
                nc.vector.tensor_copy(ssb[:], sums[0:1, :])
                nc.sync.dma_start(sumsO[m], ssb[:])

            # PE warm-up: ~3.4us of matmul activity so the HAM clock gate
            # opens before the first real projections (output is discarded).
            wt = stpool.tile([P, 2, KB], dt.float32, tag="st")
            for i in range(6):
                nc.tensor.matmul(
                    wt[:, 0, :], lhsT=wk_t[:, 0, 0, :],
                    rhs=wk_t[:, 0:2, :, :], start=True, stop=True,
                )
            wsb = ssbp.tile([1, 4], dt.float32, tag="wsb")
            nc.vector.tensor_copy(wsb[:], wt[0:1, 0, 0:4])
            nc.sync.dma_start(warmO, wsb[:])

            for j in range(NM):
                proj_q(j)
            emitted = set()
            for g in range(NG):
                proj_kv(g, xsP[g // 2][:, g % 2])
                # emit wei pairs whose kT groups are now available
                for m in range(NM):
                    npr = 4 * m + 4
                    for p in range(npr):
                        if (m, p) in emitted or (2 * p + 1) // 4 > g:
                            continue
                        emitted.add((m, p))
                        weiA(m, p)
                for m in range(NM):
                    npr = 4 * m + 4
                    if ("acc", m) not in emitted and all((m, p) in emitted for p in range(npr)):
                        emitted.add(("acc", m))
                        accum(m)

    nc.compile()
    return nc


def _qtiles_for(half):
    return [4 * (j // 2) + 2 * half + (j % 2) for j in range(16)]


def _host_prep(x, Wk, bk, Wq, bq, Wv, bv):
    scale = float(C) ** -0.5

    def tile_w(w, s):
        # [C, H] -> [P, NCP, 2, H] with c = 128*(2*cp+i)+p
        w8 = (np.asarray(w, np.float64) * s).astype(F8)
        return np.ascontiguousarray(
            w8.reshape(NCP, 2, P, H).transpose(2, 0, 1, 3)
        )

    wkqv8 = np.ascontiguousarray(np.stack([
        tile_w(Wk, WS),
        tile_w(np.asarray(Wq, np.float64) * scale, WSQ),
        tile_w(Wv, WS),
    ], axis=1))
    bq_c = (np.asarray(bq, np.float32) * scale).reshape(P, 1)
    ident = np.eye(P, dtype=F16)
    ones8 = np.ones((P, 2, P), F8)

    per_half = []
    for half in (0, 1):
        # column permutation: group g -> [my 256 | other 256]
        idx = np.empty(T, np.int64)
        for g in range(NG):
            base = KB * g
            idx[base:base + HB] = np.arange(base + HB * half, base + HB * half + HB)
            idx[base + HB:base + KB] = np.arange(
                base + HB * (1 - half), base + HB * (1 - half) + HB)
        gt = idx.reshape(NKC, P)[:, 0] // P   # permuted chunk -> global tile
        qts = _qtiles_for(half)
        m_arr = np.zeros((4, P, 2, KB), F16)
        for d in range(8):
            keys = P * gt[d] + np.arange(P)
            qrow = np.empty(KB, np.int64)
            for r in range(4):
                qrow[P * r:P * (r + 1)] = qts[r] * P + np.arange(P)
            m_arr[d // 2, :, d % 2, :] = (keys[:, None] <= qrow[None, :]).astype(F16)
        per_half.append((idx, m_arr.reshape(4, P, 2 * KB)))

    in_maps = []
    for core in range(8):
        b_idx, half = core // 2, core % 2
        idx, m_arr = per_half[half]
        xT = np.asarray(x[b_idx], np.float32).T[:, idx]     # [C, T] permuted
        xq8 = (xT * XS).astype(F8)
        x8a = xq8.reshape(NCP, 2, P, NG, 2, HB).transpose(3, 2, 4, 0, 1, 5)
        x8a = np.ascontiguousarray(
            x8a.reshape(NM, 2, P, 2, NCP, 2, HB).transpose(0, 2, 1, 3, 4, 5, 6)
        )
        in_maps.append({
            "x8": x8a, "wkqv": wkqv8,
            "bq": bq_c, "ident": ident, "ones8c": ones8, "masks": m_arr,
        })
    return in_maps


def _host_finish(x, Wk, bk, Wq, bq, Wv, bv, results):
    scale = float(C) ** -0.5
    out = np.empty((B, T, H), np.float32)
    for core in range(8):
        b_idx, half = core // 2, core % 2
        oT = np.asarray(results[core]["outT"], np.float32)      # [P, TQ]
        sums = np.asarray(results[core]["sumsO"], np.float32).reshape(TQ)
        o = oT.T / sums[:, None]
        # local col j: group g=j//256, qq=j%256 -> global t = 512g+256*half+qq
        o = o.reshape(NG, HB, H)
        for g in range(NG):
            t0 = KB * g + HB * half
            out[b_idx, t0:t0 + HB, :] = o[g]
    out += np.asarray(bv, np.float32)
    # exact repair of rows 0..RT-1 (they only attend to keys 0..2*RT-1)
    KR = 2 * RT
    xr = np.asarray(x[:, :KR, :], np.float64)
    q = xr[:, :RT] @ (np.asarray(Wq, np.float64) * scale) \
        + np.asarray(bq, np.float64) * scale
    k = xr @ np.asarray(Wk, np.float64) + np.asarray(bk, np.float64)
    v = xr @ np.asarray(Wv, np.float64) + np.asarray(bv, np.float64)
    s = np.einsum("bth,bsh->bts", q, k)
    mask = np.arange(KR)[None, :] <= np.arange(RT)[:, None]
    s = np.where(mask[None], s, -np.inf)
    s = s - s.max(-1, keepdims=True)
    e = np.exp(s)
    w = e / e.sum(-1, keepdims=True)
    out[:, :RT, :] = (np.einsum("bts,bsh->bth", w, v)).astype(np.float32)
    return out


def kernel(x, Wk, bk, Wq, bq, Wv, bv):
    if "nc" not in _NC_CACHE:
        _NC_CACHE["nc"] = build_nc()
    nc = _NC_CACHE["nc"]
    in_maps = _host_prep(x, Wk, bk, Wq, bq, Wv, bv)
    res = run_bass_kernel_spmd(nc, in_maps, list(range(8))).results
    return _host_finish(x, Wk, bk, Wq, bq, Wv, bv, res)
